# revision 1
# baseline (speedup 1.0000x reference)
"""Batched Procrustes-alignment loss on 8 Trainium2 NeuronCores.

Data-parallel over batch (B=262144 -> 32768/core), laid out as [128
partitions, F=256] planes (one scalar per batch element per plane).

Per batch element:
  center pred/target over J=17 joints; per-joint squared norms P2/T2;
  scale s = tn/(pn+eps); H = PC^T TC via streamed per-plane products +
  j-tree reductions; closed-form symmetric 3x3 eigensolver on A = H^T H
  (trigonometric eigenvalues via acos/cos expressed with Arctan+Sin
  activations, eigenvectors via cross-of-rows, v2 = v0 x v1 so det(V)=+1
  and all LAPACK sign bookkeeping cancels); u_i = H v_i / sigma_i,
  u2 = (u0 x u1)/s; G = sum_i u_i (x) m_i scaled by -2s.
  dist_j^2 = s^2 P2_j - 2 s W_j + T2_j with W via rotated-pred planes
  e_r = sum_c Gt_cr PC_c, then d2 += TC_r*e_r; loss = mean sqrt.

Output: per-core per-partition partial sums [128,1]; host sums in float64
and divides by B*J.
"""
import numpy as np
import concourse.bass as bass
import concourse.mybir as mybir
import concourse.tile as tile
from concourse import bacc
from concourse.bass_utils import run_bass_kernel_spmd

AF = mybir.ActivationFunctionType
OP = mybir.AluOpType
AX = mybir.AxisListType
f32 = mybir.dt.float32
bf16 = mybir.dt.bfloat16

B, J, C = 262144, 17, 3
JC = J * C
NCORES = 8
BC = B // NCORES
P = 128
F = 256
JF = J * F
SUB = 64
NSUB = F // SUB
EPS = 1e-8
TINY = 1e-20

# engine assignment knobs ("v" = DVE vector, "g" = gpsimd Pool, "s" = scalar/Act)
KNOBS = dict(
    center=["v", "g", "g", "g", "v", "g"],   # per (tensor*3 + c)
    omult=["v"] * 9,
    emult=["v"] * 9,
    eadd=["v"] * 6,
    tcmul=["v"] * 3,
    d2add=["v"] * 3,
    sqadd=["v", "v", "v", "v"],
    uassm="v",
    gassm="v",
    htree="v",
    sq="s",
)


def _ap(t, off, dims):
    a = t[:]
    return bass.AP(a.tensor, a.offset + off, [a.ap[0]] + dims)


def _pl(t, off, n):
    return _ap(t, off, [[1, n]])


def build_nc(iters=1, knobs=None, stop=99):
    kn = dict(KNOBS)
    if knobs:
        kn.update(knobs)

    nc = bacc.Bacc("TRN2", target_bir_lowering=False)
    nc._dbg = {}
    pred_d = nc.dram_tensor("pred", [BC, JC], f32, kind="ExternalInput")
    targ_d = nc.dram_tensor("target", [BC, JC], f32, kind="ExternalInput")
    out_d = nc.dram_tensor("partial", [P, 1], f32, kind="ExternalOutput")

    def E(key):
        v = kn[key] if isinstance(kn[key], str) else None
        assert v is not None
        return {"v": nc.vector, "g": nc.gpsimd, "s": nc.scalar}[v]

    def Ei(key, i):
        return {"v": nc.vector, "g": nc.gpsimd, "s": nc.scalar}[kn[key][i]]

    with tile.TileContext(nc) as tc:
        with (
            tc.tile_pool(name="persist", bufs=1) as persist,
            tc.tile_pool(name="rawp", bufs=1) as rawp,
            tc.tile_pool(name="meanp", bufs=1) as meanp,
            tc.tile_pool(name="pctc", bufs=1) as pctcp,
            tc.tile_pool(name="oring", bufs=1) as oring,
            tc.tile_pool(name="sqp", bufs=1) as sqp,
            tc.tile_pool(name="hp", bufs=1) as hp,
            tc.tile_pool(name="ep", bufs=1) as epool,
            tc.tile_pool(name="late", bufs=1) as late,
            tc.tile_pool(name="thinE", bufs=1) as thinE,
            tc.tile_pool(name="psth", bufs=1, space="PSUM") as psth,
        ):
            acc = persist.tile([P, F], f32, tag="acc", name="acc")
            b2p3 = persist.tile([P, 1], f32, tag="b2p3", name="b2p3")
            b4p3 = persist.tile([P, 1], f32, tag="b4p3", name="b4p3")
            nc.gpsimd.memset(acc[:], 0.0)
            nc.gpsimd.memset(b2p3[:], 2.0943951023931953)
            nc.gpsimd.memset(b4p3[:], 1.0471975511965976)  # pi/3

            def thinE_t():
                return thinE.tile([P, F], f32, tag="te", name="te", bufs=8)

            _ps = {"n": 0, "banks": []}

            def psum_t(tg):
                i = _ps["n"]
                _ps["n"] += 1
                assert i < 16
                if i % 2 == 0:
                    _ps["banks"].append(
                        psth.tile([P, 2 * F], f32, tag=f"pb{i // 2}",
                                  name=f"pb{i // 2}"))
                blk = _ps["banks"][i // 2]
                off = (i % 2) * F

                class _T:
                    def __getitem__(self, _):
                        return _pl(blk, off, F)
                return _T()

            def body():
                _ps["n"] = 0
                _ps["banks"] = []
                # --------- tiles (allocated per iteration; tags reuse slots)
                PC = pctcp.tile([P, 3 * JF], bf16, tag="PC", name="PC")
                TC = pctcp.tile([P, 3 * JF], bf16, tag="TC", name="TC")
                P2 = sqp.tile([P, JF], bf16, tag="P2", name="P2")
                T2 = sqp.tile([P, JF], bf16, tag="T2", name="T2")
                H = hp.tile([P, 9 * F], f32, tag="H", name="H")
                d2 = late.tile([P, JF], bf16, tag="d2", name="d2")
                G = late.tile([P, 9 * F], bf16, tag="G", name="G")
                mtmp = meanp.tile([P, 2304], f32, tag="mt", name="mtmp")
                mean_p = meanp.tile([P, 768], f32, tag="mp", name="mean_p")
                mean_t = meanp.tile([P, 768], f32, tag="mq", name="mean_t")

                # --------- per-sub-block load + mean tree + center
                for ti, (dram, mean, ctr) in enumerate(
                        ((pred_d, mean_p, PC), (targ_d, mean_t, TC))):
                    for s in range(NSUB):
                        raw = rawp.tile([P, JC * SUB], f32, tag="raw",
                                        name="raw", bufs=2)
                        off = (s * SUB) * JC
                        nc.sync.dma_start(
                            raw[:], bass.AP(dram[:].tensor, off,
                                            [[F * JC, P], [1, JC * SUB]]))
                        # mean tree over j (all 3 c at once); u = j*3+c
                        r1, r2, r3, r4 = 0, 1536, 0, 512
                        nc.vector.tensor_tensor(
                            _ap(mtmp, r1, [[24, SUB], [1, 24]]),
                            _ap(raw, 0, [[JC, SUB], [1, 24]]),
                            _ap(raw, 24, [[JC, SUB], [1, 24]]), OP.add)
                        nc.vector.tensor_tensor(
                            _ap(mtmp, r2, [[12, SUB], [1, 12]]),
                            _ap(mtmp, r1, [[24, SUB], [1, 12]]),
                            _ap(mtmp, r1 + 12, [[24, SUB], [1, 12]]), OP.add)
                        nc.vector.tensor_tensor(
                            _ap(mtmp, r3, [[6, SUB], [1, 6]]),
                            _ap(mtmp, r2, [[12, SUB], [1, 6]]),
                            _ap(mtmp, r2 + 6, [[12, SUB], [1, 6]]), OP.add)
                        nc.vector.tensor_tensor(
                            _ap(mtmp, r4, [[3, SUB], [1, 3]]),
                            _ap(mtmp, r3, [[6, SUB], [1, 3]]),
                            _ap(mtmp, r3 + 3, [[6, SUB], [1, 3]]), OP.add)
                        nc.vector.tensor_tensor(
                            _ap(mean, s * SUB * 3, [[3, SUB], [1, 3]]),
                            _ap(mtmp, r4, [[3, SUB], [1, 3]]),
                            _ap(raw, 48, [[JC, SUB], [1, 3]]), OP.add)
                        nc.vector.tensor_scalar_mul(
                            _ap(mean, s * SUB * 3, [[3, SUB], [1, 3]]),
                            _ap(mean, s * SUB * 3, [[3, SUB], [1, 3]]), 1.0 / J)
                        # center: PC_c[j, f] = raw - mean, per c
                        for c in range(3):
                            Ei("center", ti * 3 + c).tensor_tensor(
                                _ap(ctr, c * JF + s * SUB, [[F, J], [1, SUB]]),
                                _ap(raw, c, [[3, J], [JC, SUB]]),
                                _ap(mean, s * SUB * 3 + c, [[0, J], [3, SUB]]),
                                OP.subtract)

                # long-lived thin planes carved out of the e-phase slots:
                # they die at G-assembly, exactly when the e tiles are born,
                # so the raw DMA ring stays decoupled from the SVD tail and
                # iteration k+1's loads overlap iteration k's distance phase.
                tbs_named = [
                    epool.tile([P, 2048], f32, tag="e", name=f"tbn{i}", bufs=3)
                    for i in range(3)
                ]
                nb = {"n": 0}

                def named(tg):
                    i = nb["n"]
                    nb["n"] += 1
                    assert i < 24
                    blk = tbs_named[i // 8]
                    off = (i % 8) * F

                    class _T:
                        def __getitem__(self, _):
                            return _pl(blk, off, F)
                    return _T()

                def cblk(t, c):
                    return _pl(t, c * JF, JF)

                if stop <= 0:
                    return

                # --------- squares -> P2/T2 (Act) + adds (DVE)
                sq1 = sqp.tile([P, JF], bf16, tag="sq", name="sq1", bufs=2)
                nc.scalar.activation(P2[:], cblk(PC, 0), AF.Square)
                nc.scalar.activation(sq1[:], cblk(PC, 1), AF.Square)
                Ei("sqadd", 0).tensor_tensor(P2[:], P2[:], sq1[:], OP.add)
                sq2 = sqp.tile([P, JF], bf16, tag="sq", name="sq2", bufs=2)
                nc.scalar.activation(sq2[:], cblk(PC, 2), AF.Square)
                Ei("sqadd", 1).tensor_tensor(P2[:], P2[:], sq2[:], OP.add)
                nc.scalar.activation(T2[:], cblk(TC, 0), AF.Square)
                sq3 = sqp.tile([P, JF], bf16, tag="sq", name="sq3", bufs=2)
                nc.scalar.activation(sq3[:], cblk(TC, 1), AF.Square)
                Ei("sqadd", 2).tensor_tensor(T2[:], T2[:], sq3[:], OP.add)
                sq4 = sqp.tile([P, JF], bf16, tag="sq", name="sq4", bufs=2)
                nc.scalar.activation(sq4[:], cblk(TC, 2), AF.Square)
                Ei("sqadd", 3).tensor_tensor(T2[:], T2[:], sq4[:], OP.add)
                # sqrt planes for norms
                sp2 = sqp.tile([P, JF], bf16, tag="sq", name="sp2", bufs=2)
                nc.scalar.activation(sp2[:], P2[:], AF.Sqrt)
                st2 = sqp.tile([P, JF], bf16, tag="sq", name="st2", bufs=2)
                nc.scalar.activation(st2[:], T2[:], AF.Sqrt)

                # --------- O products (streamed) + H j-trees
                # H plane (c*3+r) = sum_j PC_c[j] * TC_r[j]
                for cc in range(3):
                    for r in range(3):
                        h = cc * 3 + r
                        O = oring.tile([P, JF], bf16, tag="O", name="O", bufs=1)
                        Ei("omult", h).tensor_tensor(
                            O[:], cblk(PC, cc), cblk(TC, r), OP.mult)
                        ht = hp.tile([P, 8 * F], f32, tag="ht", name="ht", bufs=1)
                        eng = E("htree")
                        eng.tensor_tensor(
                            ht[:], _ap(O, 0, [[F, 8], [1, F]]),
                            _ap(O, 8 * F, [[F, 8], [1, F]]), OP.add)
                        eng.tensor_tensor(
                            _pl(ht, 0, 4 * F), _pl(ht, 0, 4 * F),
                            _pl(ht, 4 * F, 4 * F), OP.add)
                        eng.tensor_tensor(
                            _pl(ht, 0, 2 * F), _pl(ht, 0, 2 * F),
                            _pl(ht, 2 * F, 2 * F), OP.add)
                        eng.tensor_tensor(
                            _pl(ht, 0, F), _pl(ht, 0, F), _pl(ht, F, F), OP.add)
                        eng.tensor_tensor(
                            _pl(H, h * F, F), _pl(ht, 0, F),
                            _pl(O, 16 * F, F), OP.add)

                def Hp(r, cc):
                    return _pl(H, (cc * 3 + r) * F, F)

                # --------- norm trees (pn from sp2, tn from st2)
                def ntree(srcpl, out):
                    ht = hp.tile([P, 8 * F], f32, tag="ht", name="nt", bufs=1)
                    nc.vector.tensor_tensor(
                        ht[:], _ap(srcpl, 0, [[F, 8], [1, F]]),
                        _ap(srcpl, 8 * F, [[F, 8], [1, F]]), OP.add)
                    nc.vector.tensor_tensor(
                        _pl(ht, 0, 4 * F), _pl(ht, 0, 4 * F),
                        _pl(ht, 4 * F, 4 * F), OP.add)
                    nc.vector.tensor_tensor(
                        _pl(ht, 0, 2 * F), _pl(ht, 0, 2 * F),
                        _pl(ht, 2 * F, 2 * F), OP.add)
                    nc.vector.tensor_tensor(
                        _pl(ht, 0, F), _pl(ht, 0, F), _pl(ht, F, F), OP.add)
                    nc.vector.tensor_tensor(
                        out[:], _pl(ht, 0, F), _pl(srcpl, 16 * F, F), OP.add)

                if stop <= 1:
                    return
                # --------- A = H^T H (6 upper entries), thin
                A6 = {}
                for (a, b) in ((0, 0), (0, 1), (0, 2), (1, 1), (1, 2), (2, 2)):
                    t1 = thinE_t()
                    nc.vector.tensor_tensor(t1[:], Hp(a, 0), Hp(b, 0), OP.mult)
                    t2 = thinE_t()
                    nc.vector.tensor_tensor(t2[:], Hp(a, 1), Hp(b, 1), OP.mult)
                    nc.vector.tensor_tensor(t1[:], t1[:], t2[:], OP.add)
                    t3 = thinE_t()
                    nc.vector.tensor_tensor(t3[:], Hp(a, 2), Hp(b, 2), OP.mult)
                    At = named(f"A{a}{b}")
                    nc.vector.tensor_tensor(At[:], t1[:], t3[:], OP.add)
                    A6[(a, b)] = At
                a00, a01, a02 = A6[(0, 0)], A6[(0, 1)], A6[(0, 2)]
                a11, a12, a22 = A6[(1, 1)], A6[(1, 2)], A6[(2, 2)]

                # --------- eigenvalues (closed form)
                q3 = thinE_t()
                nc.vector.tensor_tensor(q3[:], a00[:], a11[:], OP.add)
                nc.vector.tensor_tensor(q3[:], q3[:], a22[:], OP.add)
                m01, g0, g1 = named("m01"), named("g0"), named("g1")
                g2 = named("g2")
                nc.vector.tensor_tensor(m01[:], a01[:], a01[:], OP.mult)
                nc.vector.tensor_tensor(g0[:], a01[:], a12[:], OP.mult)
                nc.vector.tensor_tensor(g1[:], a01[:], a02[:], OP.mult)
                nc.vector.tensor_tensor(g2[:], a02[:], a12[:], OP.mult)
                m02 = thinE_t()
                nc.vector.tensor_tensor(m02[:], a02[:], a02[:], OP.mult)
                m12 = thinE_t()
                nc.vector.tensor_tensor(m12[:], a12[:], a12[:], OP.mult)
                p1 = thinE_t()
                nc.vector.tensor_tensor(p1[:], m01[:], m02[:], OP.add)
                nc.vector.tensor_tensor(p1[:], p1[:], m12[:], OP.add)
                q = named("q")
                nc.vector.tensor_scalar_mul(q[:], q3[:], 1.0 / 3)
                b00, b11, b22 = thinE_t(), thinE_t(), thinE_t()
                nc.vector.tensor_tensor(b00[:], a00[:], q[:], OP.subtract)
                nc.vector.tensor_tensor(b11[:], a11[:], q[:], OP.subtract)
                nc.vector.tensor_tensor(b22[:], a22[:], q[:], OP.subtract)
                p2s = thinE_t()
                nc.vector.tensor_tensor(p2s[:], b00[:], b00[:], OP.mult)
                tb = thinE_t()
                nc.vector.tensor_tensor(tb[:], b11[:], b11[:], OP.mult)
                nc.vector.tensor_tensor(p2s[:], p2s[:], tb[:], OP.add)
                nc.vector.tensor_tensor(tb[:], b22[:], b22[:], OP.mult)
                nc.vector.tensor_tensor(p2s[:], p2s[:], tb[:], OP.add)
                nc.vector.scalar_tensor_tensor(
                    p2s[:], p1[:], 2.0, p2s[:], OP.mult, OP.add)
                pA = named("pA")
                nc.scalar.activation(pA[:], p2s[:], AF.Sqrt, scale=1.0 / 6)
                # fill: detB terms (independent of pA)
                c0 = thinE_t()
                nc.vector.tensor_tensor(c0[:], b11[:], b22[:], OP.mult)
                nc.vector.tensor_tensor(c0[:], c0[:], m12[:], OP.subtract)
                c1 = thinE_t()
                nc.vector.tensor_tensor(c1[:], a01[:], b22[:], OP.mult)
                nc.vector.tensor_tensor(c1[:], c1[:], g2[:], OP.subtract)
                c2 = thinE_t()
                nc.vector.tensor_tensor(c2[:], b11[:], a02[:], OP.mult)
                nc.vector.tensor_tensor(c2[:], g0[:], c2[:], OP.subtract)
                detB = thinE_t()
                nc.vector.tensor_tensor(detB[:], b00[:], c0[:], OP.mult)
                tdb = thinE_t()
                nc.vector.tensor_tensor(tdb[:], a01[:], c1[:], OP.mult)
                nc.vector.tensor_tensor(detB[:], detB[:], tdb[:], OP.subtract)
                nc.vector.tensor_tensor(tdb[:], a02[:], c2[:], OP.mult)
                nc.vector.tensor_tensor(detB[:], detB[:], tdb[:], OP.add)
                pinv = thinE_t()
                nc.vector.tensor_scalar_add(pinv[:], pA[:], TINY)
                nc.vector.reciprocal_approx_fast(pinv[:], pinv[:])
                p3 = thinE_t()
                nc.vector.tensor_tensor(p3[:], pinv[:], pinv[:], OP.mult)
                nc.vector.tensor_tensor(p3[:], p3[:], pinv[:], OP.mult)
                rc = thinE_t()
                nc.vector.tensor_tensor(rc[:], detB[:], p3[:], OP.mult)
                nc.vector.tensor_scalar(rc[:], rc[:], 0.5, 1.0, OP.mult, OP.min)
                nc.vector.tensor_scalar_max(rc[:], rc[:], -1.0)
                rr = thinE_t()
                nc.vector.tensor_tensor(rr[:], rc[:], rc[:], OP.mult)
                wA = thinE_t()
                nc.scalar.activation(wA[:], rr[:], AF.Sqrt, bias=1.0, scale=-1.0)
                # fill: pn tree
                pn = psum_t("pn")
                ntree(sp2, pn)
                rat = thinE_t()
                nc.vector.tensor_scalar_add(rat[:], wA[:], 1e-10)
                nc.vector.reciprocal_approx_fast(rat[:], rat[:])
                nc.vector.tensor_tensor(rat[:], rc[:], rat[:], OP.mult)
                # atan with range reduction: |x|>1 -> sign(x)*pi/2 - atan(1/x)
                a1 = thinE_t()
                nc.vector.tensor_scalar(a1[:], rat[:], 1.0, -1.0, OP.min, OP.max)
                rat2 = thinE_t()
                nc.vector.tensor_tensor(rat2[:], rat[:], rat[:], OP.mult)
                rinv = thinE_t()
                nc.vector.tensor_scalar_add(rinv[:], rat2[:], TINY)
                nc.vector.reciprocal_approx_fast(rinv[:], rinv[:])
                nc.vector.tensor_tensor(rinv[:], rat[:], rinv[:], OP.mult)
                nc.vector.tensor_scalar(rinv[:], rinv[:], 1.0, -1.0, OP.min, OP.max)
                sg = thinE_t()
                nc.vector.tensor_scalar(sg[:], rat[:], 1e10, 1.0, OP.mult, OP.min)
                nc.vector.tensor_scalar_max(sg[:], sg[:], -1.0)
                at1 = thinE_t()
                nc.scalar.activation(at1[:], a1[:], AF.Arctan)
                at2 = thinE_t()
                nc.scalar.activation(at2[:], rinv[:], AF.Arctan)
                atb = thinE_t()
                nc.vector.scalar_tensor_tensor(
                    atb[:], sg[:], 1.5707963267948966, at2[:],
                    OP.mult, OP.subtract)
                m_ = thinE_t()
                nc.vector.tensor_scalar_add(m_[:], rat2[:], -1.0)
                nc.vector.tensor_scalar(m_[:], m_[:], 1e10, 1.0, OP.mult, OP.min)
                nc.vector.tensor_scalar_max(m_[:], m_[:], 0.0)
                atn = thinE_t()
                nc.vector.tensor_tensor(atn[:], atb[:], at1[:], OP.subtract)
                nc.vector.tensor_tensor(atn[:], atn[:], m_[:], OP.mult)
                nc.vector.tensor_tensor(atn[:], atn[:], at1[:], OP.add)
                # fill: tn tree
                tn = psum_t("tn")
                ntree(st2, tn)
                cs1 = psum_t("cs1")
                nc.scalar.activation(cs1[:], atn[:], AF.Sin,
                                     bias=b2p3[:], scale=-1.0 / 3)
                cs2 = psum_t("cs2")
                nc.scalar.activation(cs2[:], atn[:], AF.Sin,
                                     bias=b4p3[:], scale=-1.0 / 3)
                # fill: s, s2, P~2 = s^2*P2 into d2 (f32), then d2 += T2
                sS = named("sS")
                nc.vector.tensor_scalar_add(sS[:], pn[:], EPS)
                nc.vector.reciprocal_approx_fast(sS[:], sS[:])
                nc.vector.tensor_tensor(sS[:], sS[:], tn[:], OP.mult)
                s2 = psum_t("s2")
                nc.vector.tensor_tensor(s2[:], sS[:], sS[:], OP.mult)
                nc.vector.tensor_tensor(
                    d2[:], P2[:], _ap(s2, 0, [[0, J], [1, F]]), OP.mult)
                nc.vector.tensor_tensor(d2[:], d2[:], T2[:], OP.add)
                lam0, lam1 = psum_t("lam0"), psum_t("lam1")
                tp = thinE_t()
                nc.vector.tensor_tensor(tp[:], pA[:], cs1[:], OP.mult)
                nc.vector.scalar_tensor_tensor(
                    lam0[:], tp[:], 2.0, q[:], OP.mult, OP.add)
                lam2 = thinE_t()
                nc.vector.tensor_tensor(tp[:], pA[:], cs2[:], OP.mult)
                nc.vector.scalar_tensor_tensor(
                    lam2[:], tp[:], -2.0, q[:], OP.mult, OP.add)
                nc.vector.scalar_tensor_tensor(
                    lam1[:], q[:], 3.0, lam0[:], OP.mult, OP.subtract)
                nc.vector.tensor_tensor(lam1[:], lam1[:], lam2[:], OP.subtract)

                # --------- eigenvectors v0 (lam0), v1 (lam1); v2 = v0 x v1
                def eigvec(lam, pref):
                    vx = named(pref + "x")
                    vy = named(pref + "y")
                    vz = named(pref + "z")
                    b0 = thinE_t()
                    nc.vector.tensor_tensor(b0[:], a00[:], lam[:], OP.subtract)
                    b1 = thinE_t()
                    nc.vector.tensor_tensor(b1[:], a11[:], lam[:], OP.subtract)
                    nc.vector.tensor_tensor(vx[:], a02[:], b1[:], OP.mult)
                    nc.vector.tensor_tensor(vx[:], g0[:], vx[:], OP.subtract)
                    nc.vector.tensor_tensor(vy[:], b0[:], a12[:], OP.mult)
                    nc.vector.tensor_tensor(vy[:], g1[:], vy[:], OP.subtract)
                    nc.vector.tensor_tensor(vz[:], b0[:], b1[:], OP.mult)
                    nc.vector.tensor_tensor(vz[:], vz[:], m01[:], OP.subtract)
                    n2 = thinE_t()
                    nc.vector.tensor_tensor(n2[:], vx[:], vx[:], OP.mult)
                    t2_ = thinE_t()
                    nc.vector.tensor_tensor(t2_[:], vy[:], vy[:], OP.mult)
                    nc.vector.tensor_tensor(n2[:], n2[:], t2_[:], OP.add)
                    nc.vector.tensor_tensor(t2_[:], vz[:], vz[:], OP.mult)
                    nc.vector.tensor_tensor(n2[:], n2[:], t2_[:], OP.add)
                    ns = thinE_t()
                    nc.scalar.activation(ns[:], n2[:], AF.Sqrt)
                    nc.vector.tensor_scalar_add(ns[:], ns[:], TINY)
                    nc.vector.reciprocal_approx_fast(ns[:], ns[:])
                    nc.vector.tensor_tensor(vx[:], vx[:], ns[:], OP.mult)
                    nc.vector.tensor_tensor(vy[:], vy[:], ns[:], OP.mult)
                    nc.vector.tensor_tensor(vz[:], vz[:], ns[:], OP.mult)
                    return vx, vy, vz

                v0 = eigvec(lam0, "v0")
                v1 = eigvec(lam1, "v1")
                v2 = (named("v2x"), named("v2y"), named("v2z"))
                cr = ((1, 2), (2, 0), (0, 1))
                for r in range(3):
                    i1, i2 = cr[r]
                    t1 = thinE_t()
                    nc.vector.tensor_tensor(t1[:], v0[i1][:], v1[i2][:], OP.mult)
                    t2_ = thinE_t()
                    nc.vector.tensor_tensor(t2_[:], v0[i2][:], v1[i1][:], OP.mult)
                    nc.vector.tensor_tensor(v2[r][:], t1[:], t2_[:], OP.subtract)

                # --------- rsig_i = s / sigma_i ; u_i = H v_i * rsig_i
                rsig = []
                for i, lam in enumerate((lam0, lam1)):
                    rl = thinE_t()
                    nc.scalar.activation(rl[:], lam[:], AF.Relu)
                    sg = thinE_t()
                    nc.scalar.activation(sg[:], rl[:], AF.Sqrt)
                    nc.vector.tensor_scalar_add(sg[:], sg[:], TINY)
                    nc.vector.reciprocal_approx_fast(sg[:], sg[:])
                    rs = psum_t(f"rs{i}")
                    nc.vector.scalar_tensor_tensor(
                        rs[:], sg[:], -2.0, sS[:], OP.mult, OP.mult)
                    rsig.append(rs)

                ub = meanp.tile([P, 2304], f32, tag="mt", name="ublock")
                u0 = _ap(ub, 0, [[F, 3], [1, F]])
                u1 = _ap(ub, 3 * F, [[F, 3], [1, F]])
                u2 = _ap(ub, 6 * F, [[F, 3], [1, F]])

                def up(ui, r):
                    return _pl(ub, ui * 3 * F + r * F, F)

                def bc3(t):
                    return _ap(t, 0, [[0, 3], [1, F]])

                def HCg(k):
                    # H rows group for fixed k: planes (c*3+k)... careful:
                    # u_i[r] = sum_k H[r,k] v_i[k]; H[r,k] stored plane (r*3+k)?
                    # H plane (c*3+r) = H_cr = sum_j PC_c TC_r -> H[c,r].
                    # reference H_ik = sum_j pc_{j,i} tc_{j,k} -> H[i,k] = plane(i*3+k)
                    # u_i[r] = sum_k H[r,k] (v_i)_k: planes (r*3+k), r varies
                    # group for fixed k over r: offset k*F stride 3F
                    return _ap(H, k * F, [[3 * F, 3], [1, F]])

                uga = {"v": nc.vector, "g": nc.gpsimd}[kn["uassm"]]
                for i, (vv, rs) in enumerate(((v0, rsig[0]), (v1, rsig[1]))):
                    udst = (u0, u1)[i]
                    gt = meanp.tile([P, 768], f32, tag="mp", name="gt", bufs=1)
                    uga.tensor_tensor(udst, HCg(0), bc3(vv[0]), OP.mult)
                    uga.tensor_tensor(gt[:], HCg(1), bc3(vv[1]), OP.mult)
                    uga.tensor_tensor(udst, udst, gt[:], OP.add)
                    uga.tensor_tensor(gt[:], HCg(2), bc3(vv[2]), OP.mult)
                    uga.tensor_tensor(udst, udst, gt[:], OP.add)
                    uga.tensor_tensor(udst, udst, bc3(rs), OP.mult)
                # u2 = cross(u0, u1) / s
                invs = psum_t("invs")
                nc.vector.tensor_scalar_add(invs[:], sS[:], TINY)
                nc.vector.reciprocal_approx_fast(invs[:], invs[:])
                nc.vector.tensor_scalar_mul(invs[:], invs[:], -0.5)
                for r in range(3):
                    i1, i2 = cr[r]
                    t1 = thinE_t()
                    nc.vector.tensor_tensor(t1[:], up(0, i1), up(1, i2), OP.mult)
                    t2_ = thinE_t()
                    nc.vector.tensor_tensor(t2_[:], up(0, i2), up(1, i1), OP.mult)
                    nc.vector.tensor_tensor(t1[:], t1[:], t2_[:], OP.subtract)
                    nc.vector.tensor_tensor(up(2, r), t1[:], invs[:], OP.mult)

                # --------- G: plane (c*3+r) = sum_i u_i[r] * (v_c)_i, then *-2
                gga = {"v": nc.vector, "g": nc.gpsimd}[kn["gassm"]]
                vs = (v0, v1, v2)
                greps = {}
                for cc in range(3):
                    Gc = _ap(G, cc * 3 * F, [[F, 3], [1, F]])
                    gt = meanp.tile([P, 768], f32, tag="mp", name="gt2", bufs=1)
                    gt2 = meanp.tile([P, 768], f32, tag="mq", name="gt3", bufs=1)
                    gga.tensor_tensor(gt[:], u0, bc3(vs[cc][0]), OP.mult)
                    gga.tensor_tensor(gt2[:], u1, bc3(vs[cc][1]), OP.mult)
                    gga.tensor_tensor(gt[:], gt[:], gt2[:], OP.add)
                    gga.tensor_tensor(gt2[:], u2, bc3(vs[cc][2]), OP.mult)
                    gga.tensor_tensor(Gc, gt[:], gt2[:], OP.add)
                    # replicate this block's planes (r=cc, c=0..2) over j via DMA
                    for c_ in range(3):
                        gr = (oring.tile([P, JF], bf16, tag="O", name="gr",
                                         bufs=1) if c_ == 0 else
                              sqp.tile([P, JF], bf16, tag="sq", name="gr",
                                       bufs=2))
                        nc.sync.dma_start(
                            gr[:], _ap(G, (cc * 3 + c_) * F, [[0, J], [1, F]]))
                        greps[(c_, cc)] = gr

                if stop <= 2:
                    return
                # --------- e_r = sum_c Gt[c*3+r] (bcast over j) * PC_c
                def Gb(cc, r):
                    # G plane (a*3+b) holds (U M)_{b,a}; e_r needs (U M)_{cc,r}
                    if kn.get("edummy"):
                        return cblk(TC, cc)  # timing-only: plain operand
                    return _ap(G, (r * 3 + cc) * F, [[0, J], [1, F]])

                # plain mults from DMA-replicated G planes
                for r in range(3):
                    er = epool.tile([P, JF], bf16, tag="e", name="er", bufs=3)
                    tmp = epool.tile([P, JF], bf16, tag="e", name="etmp", bufs=3)
                    nc.vector.tensor_tensor(
                        er[:], cblk(PC, 0), greps[(0, r)][:], OP.mult)
                    nc.vector.tensor_tensor(
                        tmp[:], cblk(PC, 1), greps[(1, r)][:], OP.mult)
                    nc.vector.tensor_tensor(er[:], er[:], tmp[:], OP.add)
                    nc.vector.tensor_tensor(
                        tmp[:], cblk(PC, 2), greps[(2, r)][:], OP.mult)
                    nc.vector.tensor_tensor(er[:], er[:], tmp[:], OP.add)
                    Ei("tcmul", r).tensor_tensor(
                        er[:], er[:], cblk(TC, r), OP.mult)
                    Ei("d2add", r).tensor_tensor(d2[:], d2[:], er[:], OP.add)

                if stop <= 3:
                    return
                # --------- dist = sqrt(relu(d2)); sum over j; accumulate
                dr = sqp.tile([P, JF], bf16, tag="sq", name="dr", bufs=2)
                nc.scalar.activation(dr[:], d2[:], AF.Relu)
                nc.scalar.activation(dr[:], dr[:], AF.Sqrt)
                dsum = thinE_t()
                ntree(dr, dsum)
                nc.vector.tensor_tensor(acc[:], acc[:], dsum[:], OP.add)

            if iters == 1:
                body()
            else:
                with tc.For_i(0, iters, 1):
                    body()

            accs = persist.tile([P, 1], f32, tag="accs", name="accs")
            nc.vector.tensor_reduce(accs[:], acc[:], axis=AX.X, op=OP.add)
            nc.sync.dma_start(out_d[:], accs[:])

    nc.compile()
    return nc


_nc_cache = None


def get_nc():
    global _nc_cache
    if _nc_cache is None:
        _nc_cache = build_nc()
    return _nc_cache


def run(nc, pred, target, trace=False, **kw):
    pred2 = np.ascontiguousarray(np.asarray(pred), np.float32).reshape(B, JC)
    targ2 = np.ascontiguousarray(np.asarray(target), np.float32).reshape(B, JC)
    in_maps = [
        {"pred": pred2[c * BC:(c + 1) * BC], "target": targ2[c * BC:(c + 1) * BC]}
        for c in range(NCORES)
    ]
    res = run_bass_kernel_spmd(nc, in_maps, list(range(NCORES)), trace=trace, **kw)
    total = sum(r["partial"].astype(np.float64).sum() for r in res.results)
    loss = np.float32(total / (B * J))
    return loss, res


def kernel(pred, target):
    loss, _ = run(get_nc(), pred, target)
    return loss



# revision 12
# speedup vs baseline: 1.0437x; 1.0437x over previous
"""Batched Procrustes-alignment loss on 8 Trainium2 NeuronCores.

Data-parallel over batch (B=262144 -> 32768/core), laid out as [128
partitions, F=256] planes (one scalar per batch element per plane).

v2 pipeline (per core, per For_i iteration):
  DMA raw [P, 51*SUB] f32 sub-chunks; Act de-interleaves+casts to bf16
  component planes [P, 3*JF] (PC/TC after in-place centering). DVE fused-3
  bf16 j-trees give means; fused-3 in-place centered subtract.
  Squares via Act into small ping-pong scratch; folds -> P2/T2 bf16;
  Act sqrt -> sp2st2 (work slot W1); fused-2 j-tree -> pn/tn (PSUM);
  s = tn/(pn+eps) early; d2 = s^2*P2 + T2 seeded before the SVD tail.
  H phase: per r-group one fused-3 product TC_r*PC into work slot W2 and
  a fused-3 bf16 j-tree -> H[r*3+c] = H_{c,r} (f32).
  Closed-form 3x3 eigensolver on A = H^T H (A6 in PSUM, trig eigenvalues,
  eigvecs via cross-of-rows, u_i = H v_i * (-2s/sigma_i), u2 = cross/-2s);
  G' plane (r*3+c) = sum_i u_i[c] v_i[r] (-2s folded), assembled in bf16
  carved out of W1.
  e-phase (no stored O): E_acc[r] = sum_c PC_c * G'_{c,r} (bcast over j),
  then Et = E_acc * TC, folded into d2. dist = sqrt(max(d2,0)); j-tree
  sum; acc += dsum. Host sums [P,1] partials in float64.
"""
import numpy as np
import concourse.bass as bass
import concourse.mybir as mybir
import concourse.tile as tile
from concourse import bacc
from concourse.bass_utils import run_bass_kernel_spmd

AF = mybir.ActivationFunctionType
OP = mybir.AluOpType
AX = mybir.AxisListType
f32 = mybir.dt.float32
bf16 = mybir.dt.bfloat16

B, J, C = 262144, 17, 3
JC = J * C
NCORES = 8
BC = B // NCORES
P = 128
F = 256
JF = J * F
SUB = 32
NSUB = F // SUB
EPS = 1e-8
TINY = 1e-20

# engine assignment knobs ("v" = DVE vector, "g" = gpsimd Pool, "s" = scalar/Act)
KNOBS = dict(
    deint=["s"] * 16,          # per (tensor*NSUB + sub)
    meantree=["v", "v"],      # per tensor
    center=["v", "v"],        # per tensor
    p2fold=["g", "g", "g", "g"],   # P2+=sq1, P2+=sq2, T2+=sq1, T2+=sq2
    pntree="v",
    d2seed=["g", "g"],        # d2 = P2*s2, d2 += T2
    oprod=["v", "v", "v"],    # per r
    htree=["v", "v", "v"],    # per r
    eprod=["v", "v", "v"],    # per c
    eacc=["v", "v"],
    emul="v",
    wd2=["v", "g", "v"],      # three JF folds into d2
    d2max="v",
    dsum="v",
)


def _ap(t, off, dims):
    a = t[:]
    return bass.AP(a.tensor, a.offset + off, [a.ap[0]] + dims)


def _pl(t, off, n):
    return _ap(t, off, [[1, n]])


def build_nc(iters=1, knobs=None, stop=99, tap=None):
    kn = dict(KNOBS)
    if knobs:
        kn.update(knobs)
    TAP_SHAPES = dict(means=6 * F, PC=3 * JF, TC=3 * JF, P2T2=2 * JF,
                      pntn=2 * F, H=9 * F, G=9 * F, d2=JF)

    nc = bacc.Bacc("TRN2", target_bir_lowering=False)
    pred_d = nc.dram_tensor("pred", [BC, JC], f32, kind="ExternalInput")
    targ_d = nc.dram_tensor("target", [BC, JC], f32, kind="ExternalInput")
    out_d = nc.dram_tensor("partial", [P, 1], f32, kind="ExternalOutput")
    dbg_d = (nc.dram_tensor("dbg", [P, TAP_SHAPES[tap]],
                            f32 if tap in ("H", "pntn") else bf16,
                            kind="ExternalOutput") if tap else None)

    def E(key, i=None):
        v = kn[key] if i is None else kn[key][i]
        return {"v": nc.vector, "g": nc.gpsimd, "s": nc.scalar}[v]

    with tile.TileContext(nc) as tc:
        with (
            tc.tile_pool(name="persist", bufs=1) as persist,
            tc.tile_pool(name="rawp", bufs=1) as rawp,
            tc.tile_pool(name="pctc", bufs=1) as pctcp,
            tc.tile_pool(name="work", bufs=1) as work,
            tc.tile_pool(name="hp", bufs=1) as hp,
            tc.tile_pool(name="late", bufs=1) as late,
            tc.tile_pool(name="thinE", bufs=1) as thinE,
            tc.tile_pool(name="psth", bufs=1, space="PSUM") as psth,
        ):
            acc = persist.tile([P, F], f32, tag="acc", name="acc")
            b2p3 = persist.tile([P, 1], f32, tag="b2p3", name="b2p3")
            b4p3 = persist.tile([P, 1], f32, tag="b4p3", name="b4p3")
            nc.gpsimd.memset(acc[:], 0.0)
            nc.gpsimd.memset(b2p3[:], 2.0943951023931953)  # 2pi/3
            nc.gpsimd.memset(b4p3[:], 1.0471975511965976)  # pi/3

            def thinE_t():
                return thinE.tile([P, F], f32, tag="te", name="te", bufs=12)

            def named(tg):
                return thinE.tile([P, F], f32, tag="An", name=tg, bufs=22)

            _ps = {"n": 0, "banks": []}

            def psum_t(tg):
                i = _ps["n"]
                _ps["n"] += 1
                assert i < 16
                if i % 2 == 0:
                    _ps["banks"].append(
                        psth.tile([P, 2 * F], f32, tag=f"pb{i // 2}",
                                  name=f"pb{i // 2}"))
                blk = _ps["banks"][i // 2]
                off = (i % 2) * F

                class _T:
                    def __getitem__(self, _):
                        return _pl(blk, off, F)
                return _T()

            def body():
                _ps["n"] = 0
                _ps["banks"] = []
                # --------- persistent-ish tiles for this iteration
                PC = pctcp.tile([P, 3 * JF], bf16, tag="PC", name="PC")
                TC = pctcp.tile([P, 3 * JF], bf16, tag="TC", name="TC")
                means = pctcp.tile([P, 6 * F], bf16, tag="mn", name="means")
                ht = hp.tile([P, 3 * 8 * F], bf16, tag="ht", name="ht")
                H = hp.tile([P, 9 * F], f32, tag="H", name="H")
                d2 = late.tile([P, JF], bf16, tag="d2", name="d2")
                P2T2 = late.tile([P, 2 * JF], bf16, tag="p2", name="P2T2")
                Gp = late.tile([P, 9 * F], bf16, tag="G", name="Gp")

                def g3(t, off, inner=F):
                    return _ap(t, off, [[JF, 3], [F, J], [1, inner]])

                # --------- load + Act de-interleave/cast + mean + center
                for ti, (dram, ctr) in enumerate(((pred_d, PC), (targ_d, TC))):
                    for s in range(NSUB):
                        raw = rawp.tile([P, JC * SUB], f32, tag="raw",
                                        name="raw", bufs=2)
                        off = (s * SUB) * JC
                        nc.sync.dma_start(
                            raw[:], bass.AP(dram[:].tensor, off,
                                            [[F * JC, P], [1, JC * SUB]]))
                        # ctr[c][j][s*SUB+u] = raw[u*JC + j*3 + c]
                        E("deint", ti * NSUB + s).activation(
                            _ap(ctr, s * SUB, [[JF, 3], [F, J], [1, SUB]]),
                            _ap(raw, 0, [[1, 3], [3, J], [JC, SUB]]),
                            AF.Copy)
                    mn = _ap(means, ti * 3 * F, [[F, 3], [1, F]])
                    et = E("meantree", ti)
                    et.tensor_tensor(
                        _ap(ht, 0, [[8 * F, 3], [F, 8], [1, F]]),
                        _ap(ctr, 0, [[JF, 3], [F, 8], [1, F]]),
                        _ap(ctr, 8 * F, [[JF, 3], [F, 8], [1, F]]), OP.add)
                    et.tensor_tensor(
                        _ap(ht, 0, [[8 * F, 3], [F, 4], [1, F]]),
                        _ap(ht, 0, [[8 * F, 3], [F, 4], [1, F]]),
                        _ap(ht, 4 * F, [[8 * F, 3], [F, 4], [1, F]]), OP.add)
                    et.tensor_tensor(
                        _ap(ht, 0, [[8 * F, 3], [F, 2], [1, F]]),
                        _ap(ht, 0, [[8 * F, 3], [F, 2], [1, F]]),
                        _ap(ht, 2 * F, [[8 * F, 3], [F, 2], [1, F]]), OP.add)
                    et.tensor_tensor(
                        _ap(ht, 0, [[8 * F, 3], [1, F]]),
                        _ap(ht, 0, [[8 * F, 3], [1, F]]),
                        _ap(ht, F, [[8 * F, 3], [1, F]]), OP.add)
                    et.tensor_tensor(
                        mn,
                        _ap(ht, 0, [[8 * F, 3], [1, F]]),
                        _ap(ctr, 16 * F, [[JF, 3], [1, F]]), OP.add)
                    nc.vector.tensor_scalar_mul(mn, mn, 1.0 / J)
                    E("center", ti).tensor_tensor(
                        g3(ctr, 0), g3(ctr, 0),
                        _ap(means, ti * 3 * F, [[F, 3], [0, J], [1, F]]),
                        OP.subtract)

                if tap == "means":
                    nc.sync.dma_start(dbg_d[:], means[:])
                if tap == "PC":
                    nc.sync.dma_start(dbg_d[:], PC[:])
                if tap == "TC":
                    nc.sync.dma_start(dbg_d[:], TC[:])
                if stop <= 0:
                    return

                # --------- squares -> P2/T2 (Act into ht/d2 ping-pong scratch)
                P2 = _pl(P2T2, 0, JF)
                T2 = _pl(P2T2, JF, JF)
                sqh = _pl(ht, 0, JF)     # ht as JF bf16 scratch
                sqd = d2[:]              # d2 as scratch before seeding
                for ti, (ctr, dst) in enumerate(((PC, P2), (TC, T2))):
                    nc.scalar.activation(dst, _pl(ctr, 0, JF), AF.Square)
                    nc.scalar.activation(sqh, _pl(ctr, JF, JF), AF.Square)
                    nc.scalar.activation(sqd, _pl(ctr, 2 * JF, JF), AF.Square)
                    E("p2fold", ti * 2).tensor_tensor(dst, dst, sqh, OP.add)
                    E("p2fold", ti * 2 + 1).tensor_tensor(dst, dst, sqd, OP.add)

                # sqrt -> sp2st2 in work slot W1; fused-2 j-tree -> pn/tn
                W1 = work.tile([P, 3 * JF], bf16, tag="W1", name="W1a")
                sp2st2 = _pl(W1, 0, 2 * JF)
                nc.scalar.activation(sp2st2, P2T2[:], AF.Sqrt)
                pntn = psth.tile([P, 2 * F], f32, tag="pntn", name="pntn")
                et = E("pntree")
                et.tensor_tensor(
                    _ap(ht, 0, [[8 * F, 2], [F, 8], [1, F]]),
                    _ap(W1, 0, [[JF, 2], [F, 8], [1, F]]),
                    _ap(W1, 8 * F, [[JF, 2], [F, 8], [1, F]]), OP.add)
                et.tensor_tensor(
                    _ap(ht, 0, [[8 * F, 2], [F, 4], [1, F]]),
                    _ap(ht, 0, [[8 * F, 2], [F, 4], [1, F]]),
                    _ap(ht, 4 * F, [[8 * F, 2], [F, 4], [1, F]]), OP.add)
                et.tensor_tensor(
                    _ap(ht, 0, [[8 * F, 2], [F, 2], [1, F]]),
                    _ap(ht, 0, [[8 * F, 2], [F, 2], [1, F]]),
                    _ap(ht, 2 * F, [[8 * F, 2], [F, 2], [1, F]]), OP.add)
                et.tensor_tensor(
                    _ap(ht, 0, [[8 * F, 2], [1, F]]),
                    _ap(ht, 0, [[8 * F, 2], [1, F]]),
                    _ap(ht, F, [[8 * F, 2], [1, F]]), OP.add)
                et.tensor_tensor(
                    _ap(pntn, 0, [[F, 2], [1, F]]),
                    _ap(ht, 0, [[8 * F, 2], [1, F]]),
                    _ap(W1, 16 * F, [[JF, 2], [1, F]]), OP.add)
                pn = _pl(pntn, 0, F)
                tn = _pl(pntn, F, F)
                if tap == "P2T2":
                    nc.sync.dma_start(dbg_d[:], P2T2[:])
                if tap == "pntn":
                    pncp = late.tile([P, 2 * F], f32, tag="pncp", name="pncp")
                    nc.vector.tensor_copy(pncp[:], pntn[:])
                    nc.sync.dma_start(dbg_d[:], pncp[:])

                # s = tn/(pn+eps); s2 bf16 (SBUF); seed d2 = s^2*P2 + T2
                sS = named("sS")
                nc.vector.tensor_scalar_add(sS[:], pn, EPS)
                nc.vector.reciprocal_approx_fast(sS[:], sS[:])
                nc.vector.tensor_tensor(sS[:], sS[:], tn, OP.mult)
                s2b = late.tile([P, F], bf16, tag="s2b", name="s2b")
                nc.vector.tensor_tensor(s2b[:], sS[:], sS[:], OP.mult)
                E("d2seed", 0).tensor_tensor(
                    d2[:], P2, _ap(s2b, 0, [[0, J], [1, F]]), OP.mult)
                E("d2seed", 1).tensor_tensor(d2[:], d2[:], T2, OP.add)

                if stop <= 1:
                    return

                # --------- H phase: per r-group product + fused-3 bf16 j-tree
                # Op plane (r*3+c) = TC_r * PC_c ; H plane (r*3+c) = H_{c,r}
                W2 = work.tile([P, 3 * JF], bf16, tag="W2", name="W2a")
                for r in range(3):
                    E("oprod", r).tensor_tensor(
                        g3(W2, 0),
                        _ap(TC, r * JF, [[0, 3], [F, J], [1, F]]),
                        g3(PC, 0), OP.mult)
                    et = E("htree", r)
                    et.tensor_tensor(
                        _ap(ht, 0, [[8 * F, 3], [F, 8], [1, F]]),
                        _ap(W2, 0, [[JF, 3], [F, 8], [1, F]]),
                        _ap(W2, 8 * F, [[JF, 3], [F, 8], [1, F]]), OP.add)
                    et.tensor_tensor(
                        _ap(ht, 0, [[8 * F, 3], [F, 4], [1, F]]),
                        _ap(ht, 0, [[8 * F, 3], [F, 4], [1, F]]),
                        _ap(ht, 4 * F, [[8 * F, 3], [F, 4], [1, F]]), OP.add)
                    et.tensor_tensor(
                        _ap(ht, 0, [[8 * F, 3], [F, 2], [1, F]]),
                        _ap(ht, 0, [[8 * F, 3], [F, 2], [1, F]]),
                        _ap(ht, 2 * F, [[8 * F, 3], [F, 2], [1, F]]), OP.add)
                    et.tensor_tensor(
                        _ap(ht, 0, [[8 * F, 3], [1, F]]),
                        _ap(ht, 0, [[8 * F, 3], [1, F]]),
                        _ap(ht, F, [[8 * F, 3], [1, F]]), OP.add)
                    et.tensor_tensor(
                        _ap(H, r * 3 * F, [[F, 3], [1, F]]),
                        _ap(ht, 0, [[8 * F, 3], [1, F]]),
                        _ap(W2, 16 * F, [[JF, 3], [1, F]]), OP.add)

                if tap == "H":
                    nc.sync.dma_start(dbg_d[:], H[:])

                def Hp(a, cc):
                    # H_{cc,a} (pred comp cc, targ comp a) = plane (a*3+cc)
                    return _pl(H, (a * 3 + cc) * F, F)

                if stop <= 2:
                    return

                # --------- A = H^T H (6 upper entries) in PSUM, f32
                A6 = {}
                for (a, b) in ((0, 0), (0, 1), (0, 2), (1, 1), (1, 2), (2, 2)):
                    t1 = thinE_t()
                    nc.vector.tensor_tensor(t1[:], Hp(a, 0), Hp(b, 0), OP.mult)
                    t2 = thinE_t()
                    nc.vector.tensor_tensor(t2[:], Hp(a, 1), Hp(b, 1), OP.mult)
                    nc.vector.tensor_tensor(t1[:], t1[:], t2[:], OP.add)
                    t3 = thinE_t()
                    nc.vector.tensor_tensor(t3[:], Hp(a, 2), Hp(b, 2), OP.mult)
                    At = named(f"A{a}{b}")
                    nc.vector.tensor_tensor(At[:], t1[:], t3[:], OP.add)
                    A6[(a, b)] = At
                a00, a01, a02 = A6[(0, 0)], A6[(0, 1)], A6[(0, 2)]
                a11, a12, a22 = A6[(1, 1)], A6[(1, 2)], A6[(2, 2)]

                # --------- eigenvalues (closed form, f32)
                q3 = thinE_t()
                nc.vector.tensor_tensor(q3[:], a00[:], a11[:], OP.add)
                nc.vector.tensor_tensor(q3[:], q3[:], a22[:], OP.add)
                m01, g0, g1 = named("m01"), named("g0"), named("g1")
                g2 = named("g2")
                nc.vector.tensor_tensor(m01[:], a01[:], a01[:], OP.mult)
                nc.vector.tensor_tensor(g0[:], a01[:], a12[:], OP.mult)
                nc.vector.tensor_tensor(g1[:], a01[:], a02[:], OP.mult)
                nc.vector.tensor_tensor(g2[:], a02[:], a12[:], OP.mult)
                m02 = thinE_t()
                nc.vector.tensor_tensor(m02[:], a02[:], a02[:], OP.mult)
                m12 = thinE_t()
                nc.vector.tensor_tensor(m12[:], a12[:], a12[:], OP.mult)
                p1 = thinE_t()
                nc.vector.tensor_tensor(p1[:], m01[:], m02[:], OP.add)
                nc.vector.tensor_tensor(p1[:], p1[:], m12[:], OP.add)
                q = named("q")
                nc.vector.tensor_scalar_mul(q[:], q3[:], 1.0 / 3)
                b00, b11, b22 = thinE_t(), thinE_t(), thinE_t()
                nc.vector.tensor_tensor(b00[:], a00[:], q[:], OP.subtract)
                nc.vector.tensor_tensor(b11[:], a11[:], q[:], OP.subtract)
                nc.vector.tensor_tensor(b22[:], a22[:], q[:], OP.subtract)
                p2s = thinE_t()
                nc.vector.tensor_tensor(p2s[:], b00[:], b00[:], OP.mult)
                tb = thinE_t()
                nc.vector.tensor_tensor(tb[:], b11[:], b11[:], OP.mult)
                nc.vector.tensor_tensor(p2s[:], p2s[:], tb[:], OP.add)
                nc.vector.tensor_tensor(tb[:], b22[:], b22[:], OP.mult)
                nc.vector.tensor_tensor(p2s[:], p2s[:], tb[:], OP.add)
                nc.vector.scalar_tensor_tensor(
                    p2s[:], p1[:], 2.0, p2s[:], OP.mult, OP.add)
                pA = named("pA")
                nc.scalar.activation(pA[:], p2s[:], AF.Sqrt, scale=1.0 / 6)
                c0 = thinE_t()
                nc.vector.tensor_tensor(c0[:], b11[:], b22[:], OP.mult)
                nc.vector.tensor_tensor(c0[:], c0[:], m12[:], OP.subtract)
                c1 = thinE_t()
                nc.vector.tensor_tensor(c1[:], a01[:], b22[:], OP.mult)
                nc.vector.tensor_tensor(c1[:], c1[:], g2[:], OP.subtract)
                c2 = thinE_t()
                nc.vector.tensor_tensor(c2[:], b11[:], a02[:], OP.mult)
                nc.vector.tensor_tensor(c2[:], g0[:], c2[:], OP.subtract)
                detB = thinE_t()
                nc.vector.tensor_tensor(detB[:], b00[:], c0[:], OP.mult)
                tdb = thinE_t()
                nc.vector.tensor_tensor(tdb[:], a01[:], c1[:], OP.mult)
                nc.vector.tensor_tensor(detB[:], detB[:], tdb[:], OP.subtract)
                nc.vector.tensor_tensor(tdb[:], a02[:], c2[:], OP.mult)
                nc.vector.tensor_tensor(detB[:], detB[:], tdb[:], OP.add)
                pinv = thinE_t()
                nc.vector.tensor_scalar_add(pinv[:], pA[:], TINY)
                nc.vector.reciprocal_approx_fast(pinv[:], pinv[:])
                p3 = thinE_t()
                nc.vector.tensor_tensor(p3[:], pinv[:], pinv[:], OP.mult)
                nc.vector.tensor_tensor(p3[:], p3[:], pinv[:], OP.mult)
                rc = thinE_t()
                nc.vector.tensor_tensor(rc[:], detB[:], p3[:], OP.mult)
                nc.vector.tensor_scalar(rc[:], rc[:], 0.5, 1.0, OP.mult, OP.min)
                nc.vector.tensor_scalar_max(rc[:], rc[:], -1.0)
                rr = thinE_t()
                nc.vector.tensor_tensor(rr[:], rc[:], rc[:], OP.mult)
                wA = thinE_t()
                nc.scalar.activation(wA[:], rr[:], AF.Sqrt, bias=1.0, scale=-1.0)
                rat = thinE_t()
                nc.vector.tensor_scalar_add(rat[:], wA[:], 1e-10)
                nc.vector.reciprocal_approx_fast(rat[:], rat[:])
                nc.vector.tensor_tensor(rat[:], rc[:], rat[:], OP.mult)
                a1 = thinE_t()
                nc.vector.tensor_scalar(a1[:], rat[:], 1.0, -1.0, OP.min, OP.max)
                rat2 = thinE_t()
                nc.vector.tensor_tensor(rat2[:], rat[:], rat[:], OP.mult)
                rinv = thinE_t()
                nc.vector.tensor_scalar_add(rinv[:], rat2[:], TINY)
                nc.vector.reciprocal_approx_fast(rinv[:], rinv[:])
                nc.vector.tensor_tensor(rinv[:], rat[:], rinv[:], OP.mult)
                nc.vector.tensor_scalar(rinv[:], rinv[:], 1.0, -1.0, OP.min, OP.max)
                sg = thinE_t()
                nc.vector.tensor_scalar(sg[:], rat[:], 1e10, 1.0, OP.mult, OP.min)
                nc.vector.tensor_scalar_max(sg[:], sg[:], -1.0)
                at1 = thinE_t()
                nc.scalar.activation(at1[:], a1[:], AF.Arctan)
                at2 = thinE_t()
                nc.scalar.activation(at2[:], rinv[:], AF.Arctan)
                atb = thinE_t()
                nc.vector.scalar_tensor_tensor(
                    atb[:], sg[:], 1.5707963267948966, at2[:],
                    OP.mult, OP.subtract)
                m_ = thinE_t()
                nc.vector.tensor_scalar_add(m_[:], rat2[:], -1.0)
                nc.vector.tensor_scalar(m_[:], m_[:], 1e10, 1.0, OP.mult, OP.min)
                nc.vector.tensor_scalar_max(m_[:], m_[:], 0.0)
                atn = thinE_t()
                nc.vector.tensor_tensor(atn[:], atb[:], at1[:], OP.subtract)
                nc.vector.tensor_tensor(atn[:], atn[:], m_[:], OP.mult)
                nc.vector.tensor_tensor(atn[:], atn[:], at1[:], OP.add)
                cs1 = psum_t("cs1")
                nc.scalar.activation(cs1[:], atn[:], AF.Sin,
                                     bias=b2p3[:], scale=-1.0 / 3)
                cs2 = psum_t("cs2")
                nc.scalar.activation(cs2[:], atn[:], AF.Sin,
                                     bias=b4p3[:], scale=-1.0 / 3)
                lam0, lam1 = psum_t("lam0"), psum_t("lam1")
                tp = thinE_t()
                nc.vector.tensor_tensor(tp[:], pA[:], cs1[:], OP.mult)
                nc.vector.scalar_tensor_tensor(
                    lam0[:], tp[:], 2.0, q[:], OP.mult, OP.add)
                lam2 = thinE_t()
                nc.vector.tensor_tensor(tp[:], pA[:], cs2[:], OP.mult)
                nc.vector.scalar_tensor_tensor(
                    lam2[:], tp[:], -2.0, q[:], OP.mult, OP.add)
                nc.vector.scalar_tensor_tensor(
                    lam1[:], q[:], 3.0, lam0[:], OP.mult, OP.subtract)
                nc.vector.tensor_tensor(lam1[:], lam1[:], lam2[:], OP.subtract)

                # --------- eigenvectors v0, v1; v2 = v0 x v1 (f32)
                def eigvec(lam, pref):
                    vx = named(pref + "x")
                    vy = named(pref + "y")
                    vz = named(pref + "z")
                    b0 = thinE_t()
                    nc.vector.tensor_tensor(b0[:], a00[:], lam[:], OP.subtract)
                    b1 = thinE_t()
                    nc.vector.tensor_tensor(b1[:], a11[:], lam[:], OP.subtract)
                    nc.vector.tensor_tensor(vx[:], a02[:], b1[:], OP.mult)
                    nc.vector.tensor_tensor(vx[:], g0[:], vx[:], OP.subtract)
                    nc.vector.tensor_tensor(vy[:], b0[:], a12[:], OP.mult)
                    nc.vector.tensor_tensor(vy[:], g1[:], vy[:], OP.subtract)
                    nc.vector.tensor_tensor(vz[:], b0[:], b1[:], OP.mult)
                    nc.vector.tensor_tensor(vz[:], vz[:], m01[:], OP.subtract)
                    n2 = thinE_t()
                    nc.vector.tensor_tensor(n2[:], vx[:], vx[:], OP.mult)
                    t2_ = thinE_t()
                    nc.vector.tensor_tensor(t2_[:], vy[:], vy[:], OP.mult)
                    nc.vector.tensor_tensor(n2[:], n2[:], t2_[:], OP.add)
                    nc.vector.tensor_tensor(t2_[:], vz[:], vz[:], OP.mult)
                    nc.vector.tensor_tensor(n2[:], n2[:], t2_[:], OP.add)
                    ns = thinE_t()
                    nc.scalar.activation(ns[:], n2[:], AF.Sqrt)
                    nc.vector.tensor_scalar_add(ns[:], ns[:], TINY)
                    nc.vector.reciprocal_approx_fast(ns[:], ns[:])
                    nc.vector.tensor_tensor(vx[:], vx[:], ns[:], OP.mult)
                    nc.vector.tensor_tensor(vy[:], vy[:], ns[:], OP.mult)
                    nc.vector.tensor_tensor(vz[:], vz[:], ns[:], OP.mult)
                    return vx, vy, vz

                v0 = eigvec(lam0, "v0")
                v1 = eigvec(lam1, "v1")
                v2 = (named("v2x"), named("v2y"), named("v2z"))
                cr = ((1, 2), (2, 0), (0, 1))
                for r in range(3):
                    i1, i2 = cr[r]
                    t1 = thinE_t()
                    nc.vector.tensor_tensor(t1[:], v0[i1][:], v1[i2][:], OP.mult)
                    t2_ = thinE_t()
                    nc.vector.tensor_tensor(t2_[:], v0[i2][:], v1[i1][:], OP.mult)
                    nc.vector.tensor_tensor(v2[r][:], t1[:], t2_[:], OP.subtract)

                # --------- rsig_i = -2s/sigma_i
                rsig = []
                for i, lam in enumerate((lam0, lam1)):
                    rl = thinE_t()
                    nc.scalar.activation(rl[:], lam[:], AF.Relu)
                    sg_ = thinE_t()
                    nc.scalar.activation(sg_[:], rl[:], AF.Sqrt)
                    nc.vector.tensor_scalar_add(sg_[:], sg_[:], TINY)
                    nc.vector.reciprocal_approx_fast(sg_[:], sg_[:])
                    rs = psum_t(f"rs{i}")
                    nc.vector.scalar_tensor_tensor(
                        rs[:], sg_[:], -2.0, sS[:], OP.mult, OP.mult)
                    rsig.append(rs)
                invs = psum_t("invs")
                nc.vector.tensor_scalar_add(invs[:], sS[:], TINY)
                nc.vector.reciprocal_approx_fast(invs[:], invs[:])
                nc.vector.tensor_scalar_mul(invs[:], invs[:], -0.5)

                # --------- bf16 u/G assembly, carved out of W1 (sp2st2 dead)
                W1b = work.tile([P, 3 * JF], bf16, tag="W1", name="W1b")
                Hb = _pl(W1b, 0, 9 * F)                    # bf16 H copy
                vb = _pl(W1b, 9 * F, 9 * F)                # v_i[k] planes
                ub = _pl(W1b, 18 * F, 6 * F)               # u0,u1 (scaled)
                u2t = _pl(W1b, 24 * F, 3 * F)
                gt = _pl(W1b, 27 * F, 3 * F)
                gt2 = _pl(W1b, 30 * F, 3 * F)
                rsb = _pl(W1b, 33 * F, 2 * F)
                invsb = _pl(W1b, 35 * F, F)
                nc.vector.tensor_copy(Hb, H[:])
                for i, vv in enumerate((v0, v1, v2)):
                    for k in range(3):
                        nc.vector.tensor_copy(
                            _pl(W1b, (9 + i * 3 + k) * F, F), vv[k][:])
                nc.vector.tensor_copy(_pl(W1b, 33 * F, F), rsig[0][:])
                nc.vector.tensor_copy(_pl(W1b, 34 * F, F), rsig[1][:])
                nc.vector.tensor_copy(invsb, invs[:])

                def HCg(k):
                    # planes (k*3 + r) = H_{r,k}, r=0..2
                    return _ap(W1b, k * 3 * F, [[F, 3], [1, F]])

                def vbc(i, k):
                    return _ap(W1b, (9 + i * 3 + k) * F, [[0, 3], [1, F]])

                # u_i[r] = sum_k H_{r,k} (v_i)_k, scaled by rsig_i
                for i in range(2):
                    udst = _ap(W1b, (18 + i * 3) * F, [[F, 3], [1, F]])
                    nc.vector.tensor_tensor(udst, HCg(0), vbc(i, 0), OP.mult)
                    gta = _ap(W1b, 27 * F, [[F, 3], [1, F]])
                    nc.vector.tensor_tensor(gta, HCg(1), vbc(i, 1), OP.mult)
                    nc.vector.tensor_tensor(udst, udst, gta, OP.add)
                    nc.vector.tensor_tensor(gta, HCg(2), vbc(i, 2), OP.mult)
                    nc.vector.tensor_tensor(udst, udst, gta, OP.add)
                    nc.vector.tensor_tensor(
                        udst, udst, _ap(W1b, (33 + i) * F, [[0, 3], [1, F]]),
                        OP.mult)

                def up(ui, r_):
                    return _pl(W1b, (18 + ui * 3 + r_) * F, F)

                # u2 = cross(u0, u1) * (-0.5/s)
                for r_ in range(3):
                    i1, i2 = cr[r_]
                    t1b = _pl(W1b, 27 * F, F)
                    t2b = _pl(W1b, 28 * F, F)
                    nc.vector.tensor_tensor(t1b, up(0, i1), up(1, i2), OP.mult)
                    nc.vector.tensor_tensor(t2b, up(0, i2), up(1, i1), OP.mult)
                    nc.vector.tensor_tensor(t1b, t1b, t2b, OP.subtract)
                    nc.vector.tensor_tensor(
                        _pl(W1b, (24 + r_) * F, F), t1b, invsb, OP.mult)

                # --------- G' plane (r*3+c) = sum_i u_i[c] * (v_i)_r
                def ug(i):
                    base = (18 + i * 3) * F if i < 2 else 24 * F
                    return _ap(W1b, base, [[F, 3], [1, F]])

                gta = _ap(W1b, 27 * F, [[F, 3], [1, F]])
                gtb = _ap(W1b, 30 * F, [[F, 3], [1, F]])
                for r_ in range(3):
                    # G' plane (r*3+c) = sum_i u_i[c] * v_r[i]  (V^T quirk of
                    # the reference: R = Vh @ Ut, so the contraction pairs
                    # u_i with the i-th COMPONENT of v_r)
                    Grg = _ap(Gp, r_ * 3 * F, [[F, 3], [1, F]])
                    nc.vector.tensor_tensor(gta, ug(0), vbc(r_, 0), OP.mult)
                    nc.vector.tensor_tensor(gtb, ug(1), vbc(r_, 1), OP.mult)
                    nc.vector.tensor_tensor(gta, gta, gtb, OP.add)
                    nc.vector.tensor_tensor(gtb, ug(2), vbc(r_, 2), OP.mult)
                    nc.vector.tensor_tensor(Grg, gta, gtb, OP.add)

                if tap == "G":
                    nc.sync.dma_start(dbg_d[:], Gp[:])
                if stop <= 3:
                    return

                # --------- e-phase: E_acc[r] = sum_c PC_c * G'_{c,r}
                W2b = work.tile([P, 3 * JF], bf16, tag="W2", name="W2b")
                Ea = g3(W2b, 0)
                W1c = work.tile([P, 3 * JF], bf16, tag="W1", name="W1c")
                Et = g3(W1c, 0)

                def gpc(c):
                    # G' planes (r*3+c) for r=0..2: offset c*F, stride 3F
                    return _ap(Gp, c * F, [[3 * F, 3], [0, J], [1, F]])

                E("eprod", 0).tensor_tensor(
                    Ea, _ap(PC, 0, [[0, 3], [F, J], [1, F]]), gpc(0), OP.mult)
                E("eprod", 1).tensor_tensor(
                    Et, _ap(PC, JF, [[0, 3], [F, J], [1, F]]), gpc(1), OP.mult)
                E("eacc", 0).tensor_tensor(Ea, Ea, Et, OP.add)
                E("eprod", 2).tensor_tensor(
                    Et, _ap(PC, 2 * JF, [[0, 3], [F, J], [1, F]]), gpc(2),
                    OP.mult)
                E("eacc", 1).tensor_tensor(Ea, Ea, Et, OP.add)
                # Et = E_acc * TC (aligned r-planes); fold into d2
                E("emul").tensor_tensor(Et, Ea, g3(TC, 0), OP.mult)
                for c in range(3):
                    E("wd2", c).tensor_tensor(
                        d2[:], d2[:], _pl(W1c, c * JF, JF), OP.add)

                if tap == "d2":
                    nc.sync.dma_start(dbg_d[:], d2[:])
                # --------- dist = sqrt(max(d2,0)); j-tree; accumulate
                E("d2max").tensor_scalar_max(d2[:], d2[:], 0.0)
                dr = _pl(W2b, 0, JF)
                nc.scalar.activation(dr, d2[:], AF.Sqrt)
                et = E("dsum")
                et.tensor_tensor(
                    _ap(ht, 0, [[F, 8], [1, F]]),
                    _ap(W2b, 0, [[F, 8], [1, F]]),
                    _ap(W2b, 8 * F, [[F, 8], [1, F]]), OP.add)
                et.tensor_tensor(
                    _ap(ht, 0, [[F, 4], [1, F]]),
                    _ap(ht, 0, [[F, 4], [1, F]]),
                    _ap(ht, 4 * F, [[F, 4], [1, F]]), OP.add)
                et.tensor_tensor(
                    _ap(ht, 0, [[F, 2], [1, F]]),
                    _ap(ht, 0, [[F, 2], [1, F]]),
                    _ap(ht, 2 * F, [[F, 2], [1, F]]), OP.add)
                et.tensor_tensor(
                    _pl(ht, 0, F), _pl(ht, 0, F), _pl(ht, F, F), OP.add)
                et.tensor_tensor(
                    _pl(ht, 0, F), _pl(ht, 0, F), _pl(W2b, 16 * F, F), OP.add)
                nc.vector.tensor_tensor(acc[:], acc[:], _pl(ht, 0, F), OP.add)

            if iters == 1:
                body()
            else:
                with tc.For_i(0, iters, 1):
                    body()

            accs = persist.tile([P, 1], f32, tag="accs", name="accs")
            nc.vector.tensor_reduce(accs[:], acc[:], axis=AX.X, op=OP.add)
            nc.sync.dma_start(out_d[:], accs[:])

    nc.compile()
    return nc


def build_tapped(tap):
    nc = build_nc(iters=1, tap=tap)
    return nc, (lambda x: x)


_nc_cache = None


def get_nc():
    global _nc_cache
    if _nc_cache is None:
        _nc_cache = build_nc()
    return _nc_cache


def run(nc, pred, target, trace=False, **kw):
    pred2 = np.ascontiguousarray(np.asarray(pred), np.float32).reshape(B, JC)
    targ2 = np.ascontiguousarray(np.asarray(target), np.float32).reshape(B, JC)
    in_maps = [
        {"pred": pred2[c * BC:(c + 1) * BC], "target": targ2[c * BC:(c + 1) * BC]}
        for c in range(NCORES)
    ]
    res = run_bass_kernel_spmd(nc, in_maps, list(range(NCORES)), trace=trace, **kw)
    total = sum(r["partial"].astype(np.float64).sum() for r in res.results)
    loss = np.float32(total / (B * J))
    return loss, res


def kernel(pred, target):
    loss, _ = run(get_nc(), pred, target)
    return loss
